# revision 17
# baseline (speedup 1.0000x reference)
"""Bidirectional additive (Bahdanau) attention kernel for 8 TRN2 NeuronCores.

Math: score[b,i,j] = sum_a ws[a] * tanh(p1[b,i,a] + p2[b,j,a]) (+ bs, masked),
then softmax over each direction and two weighted sums.

Key trick: tanh(x+y) is approximated by a 12-term Fourier sine series
    tanh(z) ~= sum_r c_r sin(w_r z),   z in [-8, 8]   (max err 1.3e-3)
and sin(w(x+y)) = sin(wx)cos(wy) + cos(wx)sin(wy) is separable, so the whole
[L1, L2, A] tanh grid collapses into one TensorEngine matmul with K = A*2R.
Sin args are range-reduced to [-pi, pi]: the integer quotients
k = round(x*w/2pi) are precomputed on the host (int8 planes, tiny), and the
device does one fused scalar_tensor_tensor pass g = (x*s) - k per feature.
Host/device p-values agree to ~1e-7, so the reduced args stay in-domain.

Sharding: core c = 2*b + h handles batch b and L1-half h (i in [h*256,(h+1)*256)).
The softmax over L2 (w2/o2) is local; the softmax over L1 (w1/o1) needs a
pairwise AllReduce of the per-j exp-sums (2KB) and a pairwise ReduceScatter of
the partial o1 (512KB). No max-subtraction is needed: |score| <= sum|ws| ~ 4.8,
so exp() cannot overflow, and masked entries are -1e30 -> exp gives exactly 0.
"""

import numpy as np
import ml_dtypes

_BF16 = ml_dtypes.bfloat16

B, L1, L2 = 4, 512, 512
KD, A = 256, 128
VD = 256
LH = L1 // 2          # 256 rows of L1 per core
N_CORES = 8
BIGNEG = -1e30

# Fourier-sine fit of tanh on [-8, 8]: optimized frequencies with coefficient
# regularization (|c|<=1.2 so bf16 feature noise is not amplified), err 1.5e-3.
FREQS = [0.303300475, 0.600384551, 0.953011956, 1.50451129, 1.69876534,
         1.90080815, 2.13706675, 2.57046389, 3.12917619, 3.81152937]
COEFS = [1.20299812, 0.0535603648, 0.285256777, 0.0611868065, 0.0592275049,
         -0.0180370574, 0.0376890117, 0.0157671809, 0.00954604596, 0.00390899874]
RF = len(FREQS)

_PROGRAM_CACHE = {}


def _build_program():
    import concourse.bass as bass
    import concourse.tile as tile
    import concourse.mybir as mybir
    from concourse import bacc

    AF = mybir.ActivationFunctionType
    ALU = mybir.AluOpType
    F32 = mybir.dt.float32
    BF16 = mybir.dt.bfloat16
    I32 = mybir.dt.int32
    TWO_PI = float(2 * np.pi)

    nc = bacc.Bacc("TRN2", debug=False, num_devices=N_CORES)

    # ---- dram parameters (per-core shards; same names on every core) ----
    # Everything fp32 is packed into ONE [128, X] tensor (single big DMA);
    # int8 k-planes and bf16 v1 are separate params.
    dp = nc.declare_dram_parameter
    I8 = mybir.dt.int8
    # packed fp32 input: column layout (per 128-partition row):
    #  xa: LH+L2 (= [p1T half | p2T], biases included) | cws:RF | ident:128
    #  | maskbs: 2x512 | v2: 4x256
    C_HEAD = (LH + L2) + RF + 128
    C_ALL = C_HEAD + 1024 + 1024
    bigin = dp("bigin", [A, C_ALL], F32, isOutput=False)
    k8d = dp("k8", [A, 2 * RF * (LH + L2)], I8, isOutput=False)
    v1d = dp("v1b", [A, 4 * VD], mybir.dt.bfloat16, isOutput=False)
    u3d = dp("u3", [3, LH], F32, isOutput=False)
    w3d = dp("w3", [3, L2], F32, isOutput=False)
    score_o = dp("score_h", [LH, L2], F32, isOutput=True)
    w2_o = dp("w2_h", [LH, L2], F32, isOutput=True)
    w1_o = dp("w1_h", [L2, LH], F32, isOutput=True)
    o2_o = dp("o2_h", [LH, VD], F32, isOutput=True)
    o1_o = dp("o1_h", [L1, VD], F32, isOutput=True)     # full o1[b]; host slices

    groups = [[2 * b_, 2 * b_ + 1] for b_ in range(B)]

    with tile.TileContext(nc) as tc:
        with (
            tc.tile_pool(name="persist", bufs=1) as P,
            tc.tile_pool(name="feat", bufs=3) as FP,
            tc.tile_pool(name="red", bufs=3) as RP,
            tc.tile_pool(name="stage", bufs=2) as SP,
            tc.tile_pool(name="psA", bufs=1, space="PSUM") as PSA,
            tc.tile_pool(name="psB", bufs=2, space="PSUM") as PSB,
            tc.tile_pool(name="dram", bufs=1, space="DRAM") as DR,
        ):
            # ---------- load inputs (few big DMAs) ----------
            big_t = P.tile([A, C_ALL], F32, name="big_t", tag="big_t")
            # xa+cws first (feature inputs), rest second
            C1 = (LH + L2) + RF
            nc.sync.dma_start(big_t[:, 0:C1], bigin[:, 0:C1])
            nc.sync.dma_start(big_t[:, C1:C_ALL], bigin[:, C1:C_ALL])
            k8_t = P.tile([A, 2 * RF * (LH + L2)], I8, name="k8_t", tag="k8_t")
            W8 = 2 * (LH + L2)
            for lo, hi in [(0, 2), (2, 6), (6, RF)]:
                nc.scalar.dma_start(k8_t[:, lo * W8:hi * W8], k8d[:, lo * W8:hi * W8])
            v1all = P.tile([A, 4 * VD], BF16, name="v1all", tag="v1all")
            nc.scalar.dma_start(v1all[:], v1d[:])
            u3_t = P.tile([3, LH], F32, name="u3t", tag="u3t")
            nc.sync.dma_start(u3_t[:], u3d[:])
            w3_t = P.tile([3, L2], F32, name="w3t", tag="w3t")
            nc.sync.dma_start(w3_t[:], w3d[:])

            o = 0
            def view(n):
                nonlocal o
                v = big_t[:, o:o + n]
                o += n
                return v
            xa = view(LH + L2)
            cws_t = view(RF)
            ident_t = view(A)
            maskbs_t = [view(L2) for _ in range(2)]
            v2_t = [view(VD) for _ in range(4)]
            v1_t = [v1all[:, t_ * VD:(t_ + 1) * VD] for t_ in range(4)]
            k8s_t = [k8_t[:, (2 * r) * (LH + L2):(2 * r + 1) * (LH + L2)] for r in range(RF)]
            k8c_t = [k8_t[:, (2 * r + 1) * (LH + L2):(2 * r + 2) * (LH + L2)] for r in range(RF)]
            halfpi = P.tile([A, 1], F32, name="halfpi", tag="halfpi")
            nc.vector.memset(halfpi[:], float(np.pi / 2))

            # ---------- score psum tiles, accumulated over 2*RF+1 matmuls ----------
            score_ps = [PSA.tile([A, L2], F32, name=f"score_ps{i_}", tag=f"score{i_}")
                        for i_ in range(2)]

            for r in range(RF):
                s = float(FREQS[r] / TWO_PI)
                # g = (x*s) - k  in [-0.5, 0.5); one fused DVE pass per trig kind
                g_r = RP.tile([A, LH + L2], F32, tag="gred")
                nc.vector.scalar_tensor_tensor(g_r[:], xa[:], s, k8s_t[r][:],
                                               ALU.mult, ALU.subtract)
                gc_r = RP.tile([A, LH + L2], F32, tag="gcred")
                nc.vector.scalar_tensor_tensor(gc_r[:], xa[:], s, k8c_t[r][:],
                                               ALU.mult, ALU.subtract)
                sin_r = FP.tile([A, LH + L2], BF16, tag="sin")
                nc.scalar.activation(sin_r[:], g_r[:], AF.Sin, scale=TWO_PI)
                # cos(wx) = sin(2pi*(t+0.25-kc)) = sin(2pi*gc + pi/2)
                cos_r = FP.tile([A, LH + L2], BF16, tag="cos")
                nc.scalar.activation(cos_r[:], gc_r[:], AF.Sin, bias=halfpi[:], scale=TWO_PI)
                # scale x-side features by c_r * ws (per-partition vector, fp32 math)
                fs = FP.tile([A, LH], BF16, tag="fs")
                nc.vector.tensor_scalar(fs[:], sin_r[:, 0:LH], cws_t[:, r:r + 1], None, ALU.mult)
                fc = FP.tile([A, LH], BF16, tag="fc")
                nc.vector.tensor_scalar(fc[:], cos_r[:, 0:LH], cws_t[:, r:r + 1], None, ALU.mult)
                for it in range(2):
                    st = (r == 0)
                    nc.tensor.matmul(score_ps[it][:], fs[:, it * A:(it + 1) * A],
                                     cos_r[:, LH:LH + L2], start=st, stop=False)
                    nc.tensor.matmul(score_ps[it][:], fc[:, it * A:(it + 1) * A],
                                     sin_r[:, LH:LH + L2], start=False, stop=False)
            # finite additive mask (rank 3): -C*u_i - C*w_j + 2C*u_i*w_j
            for it in range(2):
                nc.tensor.matmul(score_ps[it][:], u3_t[:, it * A:(it + 1) * A],
                                 w3_t[:], start=False, stop=True)

            # ---------- exp + row stats (no max subtraction needed) ----------
            E2 = [P.tile([A, L2], F32, name=f"E2_{i_}", tag=f"E2_{i_}")
                  for i_ in range(2)]                               # [i, j] layout
            S2 = P.tile([A, 2], F32, name="S2", tag="S2")
            for it in range(2):
                nc.scalar.activation(E2[it][:], score_ps[it][:], AF.Exp,
                                     accum_out=S2[:, it:it + 1])

            # ---------- pairwise AllGather of bf16 exp matrix, ASAP ----------
            E2b = [SP.tile([A, L2], BF16, name=f"E2b{i_}", tag="E2b") for i_ in range(2)]
            for it in range(2):
                nc.vector.tensor_copy(E2b[it][:], E2[it][:])
            eIn = DR.tile([LH, L2], BF16, name="eIn", tag="eIn")
            for it in range(2):
                nc.sync.dma_start(eIn[it * A:(it + 1) * A, :], E2b[it][:])
            eAll = DR.tile([L1, L2], BF16, name="eAll", tag="eAll")
            nc.gpsimd.collective_compute(
                "AllGather", ALU.bypass, replica_groups=groups,
                ins=[eIn.opt()], outs=[eAll.opt()])
            Eall = []
            for t_ in range(4):
                t = P.tile([A, L2], BF16, name=f"Eall{t_}", tag=f"Eall{t_}")
                eng = nc.scalar if t_ % 2 == 0 else nc.sync
                eng.dma_start(t[:], eAll[t_ * A:(t_ + 1) * A, :])
                Eall.append(t)

            # ---------- local work that overlaps the collective ----------
            for it in range(2):
                so = SP.tile([A, L2], F32, tag="so")
                nc.vector.tensor_tensor(so[:], score_ps[it][:], maskbs_t[it][:], ALU.add)
                nc.sync.dma_start(score_o[it * A:(it + 1) * A, :], so[:])

            r2 = P.tile([A, 2], F32, name="r2", tag="r2")
            nc.vector.reciprocal(r2[:], S2[:])
            for it in range(2):
                w2s = SP.tile([A, L2], F32, tag="w2s")
                nc.vector.tensor_scalar(w2s[:], E2[it][:], r2[:, it:it + 1], None, ALU.mult)
                nc.sync.dma_start(w2_o[it * A:(it + 1) * A, :], w2s[:])

            E1T = [P.tile([A, LH], F32, name=f"E1T_{j_}", tag=f"E1T_{j_}")
                   for j_ in range(4)]
            for jt in range(4):
                for it in range(2):
                    tp = PSB.tile([A, A], F32, tag="tp")
                    nc.tensor.transpose(tp[:], E2[it][:, jt * A:(jt + 1) * A], ident_t[:])
                    if (jt + it) % 2 == 0:
                        nc.vector.tensor_copy(E1T[jt][:, it * A:(it + 1) * A], tp[:])
                    else:
                        nc.scalar.copy(E1T[jt][:, it * A:(it + 1) * A], tp[:])

            for it in range(2):
                o2ps = PSB.tile([A, VD], F32, name="o2ps", tag="o2ps", bufs=1)
                for jt in range(4):
                    nc.tensor.matmul(o2ps[:], E1T[jt][:, it * A:(it + 1) * A],
                                     v2_t[jt][:], start=(jt == 0), stop=(jt == 3))
                o2s = SP.tile([A, VD], F32, tag="o2s")
                nc.vector.tensor_scalar(o2s[:], o2ps[:], r2[:, it:it + 1], None, ALU.mult)
                nc.sync.dma_start(o2_o[it * A:(it + 1) * A, :], o2s[:])

            # ---------- after AllGather: S1 ([j-part] layout), w1, o1 ----------
            ones_t = P.tile([A, 1], BF16, name="ones_t", tag="ones_t")
            nc.vector.memset(ones_t[:], 1.0)
            S1 = P.tile([A, 4], F32, name="S1", tag="S1")
            for jt in range(4):
                s1ps = PSB.tile([A, 1], F32, name="s1ps", tag="s1ps", bufs=1)
                for it in range(4):
                    nc.tensor.matmul(s1ps[:], Eall[it][:, jt * A:(jt + 1) * A],
                                     ones_t[:], start=(it == 0), stop=(it == 3))
                nc.vector.tensor_copy(S1[:, jt:jt + 1], s1ps[:])
            r1 = P.tile([A, 4], F32, name="r1", tag="r1")
            nc.vector.reciprocal(r1[:], S1[:])

            for jt in range(4):
                o1ps = PSB.tile([A, VD], F32, name="o1ps", tag="o1ps", bufs=2)
                for it in range(4):
                    nc.tensor.matmul(o1ps[:], Eall[it][:, jt * A:(jt + 1) * A],
                                     v1_t[it][:], start=(it == 0), stop=(it == 3))
                o1s = SP.tile([A, VD], F32, tag="o1s")
                nc.vector.tensor_scalar(o1s[:], o1ps[:], r1[:, jt:jt + 1], None, ALU.mult)
                nc.sync.dma_start(o1_o[jt * A:(jt + 1) * A, :], o1s[:])
                w1s = SP.tile([A, LH], F32, tag="w1s")
                nc.vector.tensor_scalar(w1s[:], E1T[jt][:], r1[:, jt:jt + 1], None, ALU.mult)
                nc.scalar.dma_start(w1_o[jt * A:(jt + 1) * A, :], w1s[:])

    nc.compile()
    return nc


def _get_program():
    if "nc" not in _PROGRAM_CACHE:
        _PROGRAM_CACHE["nc"] = _build_program()
    return _PROGRAM_CACHE["nc"]


def _prep_inputs(k1, k2, v1, v2, W1, b1, W2, b2, ws, bs, k1_lengths, k2_lengths):
    f32 = np.float32
    cws_np = (np.asarray(ws, f32)[:, None] * np.asarray(COEFS, f32)[None, :]).astype(f32)
    W1f = np.asarray(W1, f32)
    W2f = np.asarray(W2, f32)
    # host copies of the projections; only the integer quotients k are derived
    # from these (an off-by-eps shifts the Sin arg by ~2pi*eps -- harmless)
    p1h = [(np.asarray(k1[b_], f32) @ W1f + np.asarray(b1, f32)).astype(f32)
           for b_ in range(B)]
    p2h = [(np.asarray(k2[b_], f32) @ W2f + np.asarray(b2, f32)).astype(f32)
           for b_ in range(B)]
    scales = (np.asarray(FREQS, f32) / f32(2 * np.pi)).astype(f32)
    in_maps = []
    for c in range(N_CORES):
        b_, h_ = c // 2, c % 2
        sl = slice(h_ * LH, (h_ + 1) * LH)
        xa_h = np.concatenate([p1h[b_][sl].T, p2h[b_].T], axis=1).astype(f32)  # [A, LH+L2]
        k8 = np.empty((A, 2 * RF * (LH + L2)), np.int8)
        W_ = LH + L2
        for r in range(RF):
            t = (xa_h * scales[r]).astype(f32)
            k8[:, (2 * r) * W_:(2 * r + 1) * W_] = np.rint(t).astype(np.int8)
            k8[:, (2 * r + 1) * W_:(2 * r + 2) * W_] = np.rint(t + f32(0.25)).astype(np.int8)
        u = (np.arange(L1)[sl] >= int(k1_lengths[b_])).astype(f32)   # [LH]
        w = (np.arange(L2) >= int(k2_lengths[b_])).astype(f32)       # [L2]
        C = f32(-BIGNEG)  # 1e30
        u3 = np.stack([-C * u, -C * np.ones_like(u), 2 * C * u]).astype(f32)
        w3 = np.stack([np.ones_like(w), w, w]).astype(f32)
        mask = (u[:, None] + w[None, :]) == 1.0
        maskbs_np = np.where(mask, f32(-np.inf), f32(bs)).astype(f32)
        # packed fp32 input [A, C_ALL]: xa|cws|ident|maskbs|v2
        v2f = np.asarray(v2[b_], f32)                # [L2, VD]
        parts = [
            xa_h,
            cws_np, np.eye(A, dtype=f32),
            maskbs_np[0:A, :], maskbs_np[A:2 * A, :],
            v2f[0:A, :], v2f[A:2 * A, :], v2f[2 * A:3 * A, :], v2f[3 * A:4 * A, :],
        ]
        bigin = np.ascontiguousarray(np.concatenate(parts, axis=1).astype(f32))
        v1f = np.asarray(v1[b_], f32)                # [L1, VD] full
        v1b = np.ascontiguousarray(np.concatenate(
            [v1f[t_ * A:(t_ + 1) * A, :] for t_ in range(4)], axis=1).astype(_BF16))
        in_maps.append({
            "bigin": bigin,
            "k8": np.ascontiguousarray(k8),
            "v1b": v1b,
            "u3": np.ascontiguousarray(u3),
            "w3": np.ascontiguousarray(w3),
        })
    return in_maps


def _execute(inputs, trace=False):
    from concourse.bass_utils import run_bass_kernel_spmd
    nc = _get_program()
    in_maps = _prep_inputs(**inputs)
    res = run_bass_kernel_spmd(nc, in_maps, list(range(N_CORES)), trace=trace)
    f32 = np.float32
    o1 = np.empty((B, L2, VD), f32)
    o2 = np.empty((B, L1, VD), f32)
    w1 = np.empty((B, L2, L1), f32)
    w2 = np.empty((B, L1, L2), f32)
    score = np.empty((B, L1, L2), f32)
    for c in range(N_CORES):
        b_, h_ = c // 2, c % 2
        sl = slice(h_ * LH, (h_ + 1) * LH)
        r = res.results[c]
        score[b_, sl] = r["score_h"]
        w2[b_, sl] = r["w2_h"]
        o2[b_, sl] = r["o2_h"]
        w1[b_, :, sl] = r["w1_h"]
        if h_ == 0:
            o1[b_] = r["o1_h"]  # both cores compute full o1; take h=0's
    return (o1, o2, w1, w2, score), res


def kernel(**inputs):
    outs, _ = _execute(inputs, trace=False)
    return outs


# revision 18
# speedup vs baseline: 1.0027x; 1.0027x over previous
"""Bidirectional additive (Bahdanau) attention kernel for 8 TRN2 NeuronCores.

Math: score[b,i,j] = sum_a ws[a] * tanh(p1[b,i,a] + p2[b,j,a]) (+ bs, masked),
then softmax over each direction and two weighted sums.

Key trick: tanh(x+y) is approximated by a 10-term Fourier sine series
    tanh(z) ~= sum_r c_r sin(w_r z),   z in [-8, 8]   (max err 1.5e-3,
    frequencies optimized with |c_r| <= 1.2 so bf16 noise is not amplified)
and sin(w(x+y)) = sin(wx)cos(wy) + cos(wx)sin(wy) is separable, so the whole
[L1, L2, A] tanh grid collapses into one TensorEngine matmul with K = A*2R.
Sin args are range-reduced to [-pi, pi]: the integer quotients
k = round(x*w/2pi) are precomputed on the host (int8 planes, tiny), and the
device does one fused scalar_tensor_tensor pass g = (x*s) - k per feature.
Host/device p-values agree to ~1e-7, so the reduced args stay in-domain.

Sharding: core c = 2*b + h handles batch b and L1-half h (i in [h*256,(h+1)*256)).
The softmax over L2 (w2/o2) is local; the softmax over L1 (w1/o1) needs a
pairwise AllReduce of the per-j exp-sums (2KB) and a pairwise ReduceScatter of
the partial o1 (512KB). No max-subtraction is needed: |score| <= sum|ws| ~ 4.8,
so exp() cannot overflow, and masked entries are -1e30 -> exp gives exactly 0.
"""

import numpy as np
import ml_dtypes

_BF16 = ml_dtypes.bfloat16

B, L1, L2 = 4, 512, 512
KD, A = 256, 128
VD = 256
LH = L1 // 2          # 256 rows of L1 per core
N_CORES = 8
BIGNEG = -1e30

# Fourier-sine fit of tanh on [-8, 8]: optimized frequencies with coefficient
# regularization (|c|<=1.2 so bf16 feature noise is not amplified), err 1.5e-3.
FREQS = [0.303300475, 0.600384551, 0.953011956, 1.50451129, 1.69876534,
         1.90080815, 2.13706675, 2.57046389, 3.12917619, 3.81152937]
COEFS = [1.20299812, 0.0535603648, 0.285256777, 0.0611868065, 0.0592275049,
         -0.0180370574, 0.0376890117, 0.0157671809, 0.00954604596, 0.00390899874]
RF = len(FREQS)

_PROGRAM_CACHE = {}


def _build_program():
    import concourse.bass as bass
    import concourse.tile as tile
    import concourse.mybir as mybir
    from concourse import bacc

    AF = mybir.ActivationFunctionType
    ALU = mybir.AluOpType
    F32 = mybir.dt.float32
    BF16 = mybir.dt.bfloat16
    I32 = mybir.dt.int32
    TWO_PI = float(2 * np.pi)

    nc = bacc.Bacc("TRN2", debug=False, num_devices=N_CORES)

    # ---- dram parameters (per-core shards; same names on every core) ----
    # Everything fp32 is packed into ONE [128, X] tensor (single big DMA);
    # int8 k-planes and bf16 v1 are separate params.
    dp = nc.declare_dram_parameter
    I8 = mybir.dt.int8
    # packed fp32 input: column layout (per 128-partition row):
    #  xa: LH+L2 (= [p1T half | p2T], biases included) | cws:RF | ident:128
    #  | maskbs: 2x512 | v2: 4x256
    C_HEAD = (LH + L2) + RF + 128
    C_ALL = C_HEAD + 1024 + 1024
    bigin = dp("bigin", [A, C_ALL], F32, isOutput=False)
    k8d = dp("k8", [A, 2 * RF * (LH + L2)], I8, isOutput=False)
    v1d = dp("v1b", [A, 4 * VD], mybir.dt.bfloat16, isOutput=False)
    u3d = dp("u3", [3, LH], F32, isOutput=False)
    w3d = dp("w3", [3, L2], F32, isOutput=False)
    score_o = dp("score_h", [LH, L2], F32, isOutput=True)
    w2_o = dp("w2_h", [LH, L2], F32, isOutput=True)
    w1_o = dp("w1_h", [L2, LH], F32, isOutput=True)
    o2_o = dp("o2_h", [LH, VD], F32, isOutput=True)
    o1_o = dp("o1_h", [L1, VD], F32, isOutput=True)     # full o1[b]; host slices

    groups = [[2 * b_, 2 * b_ + 1] for b_ in range(B)]

    with tile.TileContext(nc) as tc:
        with (
            tc.tile_pool(name="persist", bufs=1) as P,
            tc.tile_pool(name="feat", bufs=3) as FP,
            tc.tile_pool(name="red", bufs=3) as RP,
            tc.tile_pool(name="stage", bufs=2) as SP,
            tc.tile_pool(name="psA", bufs=1, space="PSUM") as PSA,
            tc.tile_pool(name="psB", bufs=2, space="PSUM") as PSB,
            tc.tile_pool(name="dram", bufs=1, space="DRAM") as DR,
        ):
            # ---------- load inputs (few big DMAs) ----------
            big_t = P.tile([A, C_ALL], F32, name="big_t", tag="big_t")
            # xa+cws first (feature inputs), rest second
            C1 = (LH + L2) + RF
            nc.sync.dma_start(big_t[:, 0:C1], bigin[:, 0:C1])
            nc.sync.dma_start(big_t[:, C1:C_ALL], bigin[:, C1:C_ALL])
            k8_t = P.tile([A, 2 * RF * (LH + L2)], I8, name="k8_t", tag="k8_t")
            W8 = 2 * (LH + L2)
            for lo, hi in [(0, 2), (2, 6), (6, RF)]:
                nc.scalar.dma_start(k8_t[:, lo * W8:hi * W8], k8d[:, lo * W8:hi * W8])
            v1all = P.tile([A, 4 * VD], BF16, name="v1all", tag="v1all")
            nc.scalar.dma_start(v1all[:], v1d[:])
            u3_t = P.tile([3, LH], F32, name="u3t", tag="u3t")
            nc.sync.dma_start(u3_t[:], u3d[:])
            w3_t = P.tile([3, L2], F32, name="w3t", tag="w3t")
            nc.sync.dma_start(w3_t[:], w3d[:])

            o = 0
            def view(n):
                nonlocal o
                v = big_t[:, o:o + n]
                o += n
                return v
            xa = view(LH + L2)
            cws_t = view(RF)
            ident_t = view(A)
            maskbs_t = [view(L2) for _ in range(2)]
            v2_t = [view(VD) for _ in range(4)]
            v1_t = [v1all[:, t_ * VD:(t_ + 1) * VD] for t_ in range(4)]
            k8s_t = [k8_t[:, (2 * r) * (LH + L2):(2 * r + 1) * (LH + L2)] for r in range(RF)]
            k8c_t = [k8_t[:, (2 * r + 1) * (LH + L2):(2 * r + 2) * (LH + L2)] for r in range(RF)]
            halfpi = P.tile([A, 1], F32, name="halfpi", tag="halfpi")
            nc.vector.memset(halfpi[:], float(np.pi / 2))

            # ---------- score psum tiles, accumulated over 2*RF+1 matmuls ----------
            score_ps = [PSA.tile([A, L2], F32, name=f"score_ps{i_}", tag=f"score{i_}")
                        for i_ in range(2)]

            for r in range(RF):
                s = float(FREQS[r] / TWO_PI)
                # g = (x*s) - k  in [-0.5, 0.5); one fused DVE pass per trig kind
                g_r = RP.tile([A, LH + L2], F32, tag="gred")
                nc.vector.scalar_tensor_tensor(g_r[:], xa[:], s, k8s_t[r][:],
                                               ALU.mult, ALU.subtract)
                gc_r = RP.tile([A, LH + L2], F32, tag="gcred")
                nc.vector.scalar_tensor_tensor(gc_r[:], xa[:], s, k8c_t[r][:],
                                               ALU.mult, ALU.subtract)
                sin_r = FP.tile([A, LH + L2], BF16, tag="sin")
                nc.scalar.activation(sin_r[:], g_r[:], AF.Sin, scale=TWO_PI)
                # cos(wx) = sin(2pi*(t+0.25-kc)) = sin(2pi*gc + pi/2)
                cos_r = FP.tile([A, LH + L2], BF16, tag="cos")
                nc.scalar.activation(cos_r[:], gc_r[:], AF.Sin, bias=halfpi[:], scale=TWO_PI)
                # scale x-side features by c_r * ws (per-partition vector, fp32 math)
                fs = FP.tile([A, LH], BF16, tag="fs")
                nc.vector.tensor_scalar(fs[:], sin_r[:, 0:LH], cws_t[:, r:r + 1], None, ALU.mult)
                fc = FP.tile([A, LH], BF16, tag="fc")
                nc.vector.tensor_scalar(fc[:], cos_r[:, 0:LH], cws_t[:, r:r + 1], None, ALU.mult)
                for it in range(2):
                    st = (r == 0)
                    nc.tensor.matmul(score_ps[it][:], fs[:, it * A:(it + 1) * A],
                                     cos_r[:, LH:LH + L2], start=st, stop=False)
                    nc.tensor.matmul(score_ps[it][:], fc[:, it * A:(it + 1) * A],
                                     sin_r[:, LH:LH + L2], start=False, stop=False)
            # finite additive mask (rank 3): -C*u_i - C*w_j + 2C*u_i*w_j
            for it in range(2):
                nc.tensor.matmul(score_ps[it][:], u3_t[:, it * A:(it + 1) * A],
                                 w3_t[:], start=False, stop=True)

            # ---------- exp + row stats (no max subtraction needed) ----------
            E2 = [P.tile([A, L2], F32, name=f"E2_{i_}", tag=f"E2_{i_}")
                  for i_ in range(2)]                               # [i, j] layout
            S2 = P.tile([A, 2], F32, name="S2", tag="S2")
            for it in range(2):
                nc.scalar.activation(E2[it][:], score_ps[it][:], AF.Exp,
                                     accum_out=S2[:, it:it + 1])

            # ---------- pairwise AllGather of bf16 exp matrix, ASAP ----------
            E2b = [SP.tile([A, L2], BF16, name=f"E2b{i_}", tag="E2b") for i_ in range(2)]
            for it in range(2):
                nc.vector.tensor_copy(E2b[it][:], E2[it][:])
            eIn = DR.tile([LH, L2], BF16, name="eIn", tag="eIn")
            for it in range(2):
                nc.sync.dma_start(eIn[it * A:(it + 1) * A, :], E2b[it][:])
            eAll = DR.tile([L1, L2], BF16, name="eAll", tag="eAll")
            nc.gpsimd.collective_compute(
                "AllGather", ALU.bypass, replica_groups=groups,
                ins=[eIn.opt()], outs=[eAll.opt()])
            Eall = []
            for t_ in range(4):
                t = P.tile([A, L2], BF16, name=f"Eall{t_}", tag=f"Eall{t_}")
                eng = nc.scalar if t_ % 2 == 0 else nc.sync
                eng.dma_start(t[:], eAll[t_ * A:(t_ + 1) * A, :])
                Eall.append(t)

            # ---------- local work that overlaps the collective ----------
            for it in range(2):
                so = SP.tile([A, L2], F32, tag="so")
                nc.vector.tensor_tensor(so[:], score_ps[it][:], maskbs_t[it][:], ALU.add)
                nc.sync.dma_start(score_o[it * A:(it + 1) * A, :], so[:])

            r2 = P.tile([A, 2], F32, name="r2", tag="r2")
            nc.vector.reciprocal(r2[:], S2[:])
            for it in range(2):
                w2s = SP.tile([A, L2], F32, tag="w2s")
                nc.vector.tensor_scalar(w2s[:], E2[it][:], r2[:, it:it + 1], None, ALU.mult)
                nc.sync.dma_start(w2_o[it * A:(it + 1) * A, :], w2s[:])

            E1T = [P.tile([A, LH], F32, name=f"E1T_{j_}", tag=f"E1T_{j_}")
                   for j_ in range(4)]
            for jt in range(4):
                for it in range(2):
                    tp = PSB.tile([A, A], F32, tag="tp")
                    nc.tensor.transpose(tp[:], E2[it][:, jt * A:(jt + 1) * A], ident_t[:])
                    if (jt + it) % 2 == 0:
                        nc.vector.tensor_copy(E1T[jt][:, it * A:(it + 1) * A], tp[:])
                    else:
                        nc.scalar.copy(E1T[jt][:, it * A:(it + 1) * A], tp[:])

            for it in range(2):
                o2ps = PSB.tile([A, VD], F32, name="o2ps", tag="o2ps", bufs=1)
                for jt in range(4):
                    nc.tensor.matmul(o2ps[:], E1T[jt][:, it * A:(it + 1) * A],
                                     v2_t[jt][:], start=(jt == 0), stop=(jt == 3))
                o2s = SP.tile([A, VD], F32, tag="o2s")
                nc.vector.tensor_scalar(o2s[:], o2ps[:], r2[:, it:it + 1], None, ALU.mult)
                nc.sync.dma_start(o2_o[it * A:(it + 1) * A, :], o2s[:])

            # ---------- after AllGather: S1 ([j-part] layout), w1, o1 ----------
            ones_t = P.tile([A, 1], BF16, name="ones_t", tag="ones_t")
            nc.vector.memset(ones_t[:], 1.0)
            S1 = P.tile([A, 4], F32, name="S1", tag="S1")
            for jt in range(4):
                s1ps = PSB.tile([A, 1], F32, name="s1ps", tag="s1ps", bufs=1)
                for it in range(4):
                    nc.tensor.matmul(s1ps[:], Eall[it][:, jt * A:(jt + 1) * A],
                                     ones_t[:], start=(it == 0), stop=(it == 3))
                nc.vector.tensor_copy(S1[:, jt:jt + 1], s1ps[:])
            r1 = P.tile([A, 4], F32, name="r1", tag="r1")
            nc.vector.reciprocal(r1[:], S1[:])

            for jt in range(4):
                o1ps = PSB.tile([A, VD], F32, name="o1ps", tag="o1ps", bufs=2)
                for it in range(4):
                    nc.tensor.matmul(o1ps[:], Eall[it][:, jt * A:(jt + 1) * A],
                                     v1_t[it][:], start=(it == 0), stop=(it == 3))
                o1s = SP.tile([A, VD], F32, tag="o1s")
                nc.vector.tensor_scalar(o1s[:], o1ps[:], r1[:, jt:jt + 1], None, ALU.mult)
                nc.sync.dma_start(o1_o[jt * A:(jt + 1) * A, :], o1s[:])
                w1s = SP.tile([A, LH], F32, tag="w1s")
                nc.vector.tensor_scalar(w1s[:], E1T[jt][:], r1[:, jt:jt + 1], None, ALU.mult)
                nc.scalar.dma_start(w1_o[jt * A:(jt + 1) * A, :], w1s[:])

    nc.compile()
    return nc


def _get_program():
    if "nc" not in _PROGRAM_CACHE:
        _PROGRAM_CACHE["nc"] = _build_program()
    return _PROGRAM_CACHE["nc"]


def _prep_inputs(k1, k2, v1, v2, W1, b1, W2, b2, ws, bs, k1_lengths, k2_lengths):
    f32 = np.float32
    cws_np = (np.asarray(ws, f32)[:, None] * np.asarray(COEFS, f32)[None, :]).astype(f32)
    W1f = np.asarray(W1, f32)
    W2f = np.asarray(W2, f32)
    # host copies of the projections; only the integer quotients k are derived
    # from these (an off-by-eps shifts the Sin arg by ~2pi*eps -- harmless)
    p1h = [(np.asarray(k1[b_], f32) @ W1f + np.asarray(b1, f32)).astype(f32)
           for b_ in range(B)]
    p2h = [(np.asarray(k2[b_], f32) @ W2f + np.asarray(b2, f32)).astype(f32)
           for b_ in range(B)]
    scales = (np.asarray(FREQS, f32) / f32(2 * np.pi)).astype(f32)
    in_maps = []
    for c in range(N_CORES):
        b_, h_ = c // 2, c % 2
        sl = slice(h_ * LH, (h_ + 1) * LH)
        xa_h = np.concatenate([p1h[b_][sl].T, p2h[b_].T], axis=1).astype(f32)  # [A, LH+L2]
        k8 = np.empty((A, 2 * RF * (LH + L2)), np.int8)
        W_ = LH + L2
        for r in range(RF):
            t = (xa_h * scales[r]).astype(f32)
            k8[:, (2 * r) * W_:(2 * r + 1) * W_] = np.rint(t).astype(np.int8)
            k8[:, (2 * r + 1) * W_:(2 * r + 2) * W_] = np.rint(t + f32(0.25)).astype(np.int8)
        u = (np.arange(L1)[sl] >= int(k1_lengths[b_])).astype(f32)   # [LH]
        w = (np.arange(L2) >= int(k2_lengths[b_])).astype(f32)       # [L2]
        C = f32(-BIGNEG)  # 1e30
        u3 = np.stack([-C * u, -C * np.ones_like(u), 2 * C * u]).astype(f32)
        w3 = np.stack([np.ones_like(w), w, w]).astype(f32)
        mask = (u[:, None] + w[None, :]) == 1.0
        maskbs_np = np.where(mask, f32(-np.inf), f32(bs)).astype(f32)
        # packed fp32 input [A, C_ALL]: xa|cws|ident|maskbs|v2
        v2f = np.asarray(v2[b_], f32)                # [L2, VD]
        parts = [
            xa_h,
            cws_np, np.eye(A, dtype=f32),
            maskbs_np[0:A, :], maskbs_np[A:2 * A, :],
            v2f[0:A, :], v2f[A:2 * A, :], v2f[2 * A:3 * A, :], v2f[3 * A:4 * A, :],
        ]
        bigin = np.ascontiguousarray(np.concatenate(parts, axis=1).astype(f32))
        v1f = np.asarray(v1[b_], f32)                # [L1, VD] full
        v1b = np.ascontiguousarray(np.concatenate(
            [v1f[t_ * A:(t_ + 1) * A, :] for t_ in range(4)], axis=1).astype(_BF16))
        in_maps.append({
            "bigin": bigin,
            "k8": np.ascontiguousarray(k8),
            "v1b": v1b,
            "u3": np.ascontiguousarray(u3),
            "w3": np.ascontiguousarray(w3),
        })
    return in_maps


def _execute(inputs, trace=False):
    from concourse.bass_utils import run_bass_kernel_spmd
    nc = _get_program()
    in_maps = _prep_inputs(**inputs)
    res = run_bass_kernel_spmd(nc, in_maps, list(range(N_CORES)), trace=trace)
    f32 = np.float32
    o1 = np.empty((B, L2, VD), f32)
    o2 = np.empty((B, L1, VD), f32)
    w1 = np.empty((B, L2, L1), f32)
    w2 = np.empty((B, L1, L2), f32)
    score = np.empty((B, L1, L2), f32)
    for c in range(N_CORES):
        b_, h_ = c // 2, c % 2
        sl = slice(h_ * LH, (h_ + 1) * LH)
        r = res.results[c]
        score[b_, sl] = r["score_h"]
        w2[b_, sl] = r["w2_h"]
        o2[b_, sl] = r["o2_h"]
        w1[b_, :, sl] = r["w1_h"]
        if h_ == 0:
            o1[b_] = r["o1_h"]  # both cores compute full o1; take h=0's
    return (o1, o2, w1, w2, score), res


def kernel(**inputs):
    outs, _ = _execute(inputs, trace=False)
    return outs


# revision 19
# speedup vs baseline: 1.1121x; 1.1091x over previous
"""Bidirectional additive (Bahdanau) attention kernel for 8 TRN2 NeuronCores.

Math: score[b,i,j] = sum_a ws[a] * tanh(p1[b,i,a] + p2[b,j,a]) (+ bs, masked),
then softmax over each direction and two weighted sums.

Key trick: tanh(x+y) is approximated by a 10-term Fourier sine series
    tanh(z) ~= sum_r c_r sin(w_r z),   z in [-8, 8]   (max err 1.5e-3,
    frequencies optimized with |c_r| <= 1.2 so bf16 noise is not amplified)
and sin(w(x+y)) = sin(wx)cos(wy) + cos(wx)sin(wy) is separable, so the whole
[L1, L2, A] tanh grid collapses into one TensorEngine matmul with K = A*2R.
Sin args are range-reduced to [-pi, pi]: the integer quotients
k = round(x*w/2pi) are precomputed on the host (int8 planes, tiny), and the
device does one fused scalar_tensor_tensor pass g = (x*s) - k per feature.
Host/device p-values agree to ~1e-7, so the reduced args stay in-domain.

Sharding: core c = 2*b + h handles batch b and L1-half h (i in [h*256,(h+1)*256)).
The softmax over L2 (w2/o2) is local; the softmax over L1 (w1/o1) needs a
pairwise AllReduce of the per-j exp-sums (2KB) and a pairwise ReduceScatter of
the partial o1 (512KB). No max-subtraction is needed: |score| <= sum|ws| ~ 4.8,
so exp() cannot overflow, and masked entries are -1e30 -> exp gives exactly 0.
"""

import numpy as np
import ml_dtypes

_BF16 = ml_dtypes.bfloat16

B, L1, L2 = 4, 512, 512
KD, A = 256, 128
VD = 256
LH = L1 // 2          # 256 rows of L1 per core
N_CORES = 8
BIGNEG = -1e30

# Fourier-sine fit of tanh on [-8, 8]: optimized frequencies with coefficient
# regularization (|c|<=1.2 so bf16 feature noise is not amplified), err 1.5e-3.
FREQS = [0.303300475, 0.600384551, 0.953011956, 1.50451129, 1.69876534,
         1.90080815, 2.13706675, 2.57046389, 3.12917619, 3.81152937]
COEFS = [1.20299812, 0.0535603648, 0.285256777, 0.0611868065, 0.0592275049,
         -0.0180370574, 0.0376890117, 0.0157671809, 0.00954604596, 0.00390899874]
RF = len(FREQS)

_PROGRAM_CACHE = {}


def _build_program():
    import concourse.bass as bass
    import concourse.tile as tile
    import concourse.mybir as mybir
    from concourse import bacc

    AF = mybir.ActivationFunctionType
    ALU = mybir.AluOpType
    F32 = mybir.dt.float32
    BF16 = mybir.dt.bfloat16
    I32 = mybir.dt.int32
    TWO_PI = float(2 * np.pi)

    nc = bacc.Bacc("TRN2", debug=False, num_devices=N_CORES)

    # ---- dram parameters (per-core shards; same names on every core) ----
    # Everything fp32 is packed into ONE [128, X] tensor (single big DMA);
    # int8 k-planes and bf16 v1 are separate params.
    dp = nc.declare_dram_parameter
    I8 = mybir.dt.int8
    # packed fp32 input: column layout (per 128-partition row):
    #  xa: LH+L2 (= [p1T half | p2T], biases included) | cws:RF | ident:128
    #  | maskbs: 2x512 | v2: 4x256
    C_HEAD = (LH + L2) + RF + 128
    C_ALL = C_HEAD + 1024 + 1024
    bigin = dp("bigin", [A, C_ALL], F32, isOutput=False)
    k8d = dp("k8", [A, 2 * RF * (LH + L2)], I8, isOutput=False)
    v1d = dp("v1b", [A, 4 * VD], mybir.dt.bfloat16, isOutput=False)
    u3d = dp("u3", [3, LH], F32, isOutput=False)
    w3d = dp("w3", [3, L2], F32, isOutput=False)
    score_o = dp("score_h", [LH, L2], F32, isOutput=True)
    w2_o = dp("w2_h", [LH, L2], F32, isOutput=True)
    w1_o = dp("w1_h", [L2, LH], F32, isOutput=True)
    o2_o = dp("o2_h", [LH, VD], F32, isOutput=True)
    o1_o = dp("o1_h", [L1, VD], F32, isOutput=True)     # full o1[b]; host slices

    groups = [[2 * b_, 2 * b_ + 1] for b_ in range(B)]

    with tile.TileContext(nc) as tc:
        with (
            tc.tile_pool(name="persist", bufs=1) as P,
            tc.tile_pool(name="feat", bufs=4) as FP,
            tc.tile_pool(name="red", bufs=4) as RP,
            tc.tile_pool(name="stage", bufs=2) as SP,
            tc.tile_pool(name="psA", bufs=1, space="PSUM") as PSA,
            tc.tile_pool(name="psB", bufs=2, space="PSUM") as PSB,
            tc.tile_pool(name="dram", bufs=1, space="DRAM") as DR,
        ):
            # ---------- load inputs (few big DMAs) ----------
            big_t = P.tile([A, C_ALL], F32, name="big_t", tag="big_t")
            # xa+cws first (feature inputs), rest second
            C1 = (LH + L2) + RF
            nc.sync.dma_start(big_t[:, 0:C1], bigin[:, 0:C1])
            nc.sync.dma_start(big_t[:, C1:C_ALL], bigin[:, C1:C_ALL])
            k8_t = P.tile([A, 2 * RF * (LH + L2)], I8, name="k8_t", tag="k8_t")
            W8 = 2 * (LH + L2)
            for lo, hi in [(0, 2), (2, 6), (6, RF)]:
                nc.scalar.dma_start(k8_t[:, lo * W8:hi * W8], k8d[:, lo * W8:hi * W8])
            v1all = P.tile([A, 4 * VD], BF16, name="v1all", tag="v1all")
            nc.scalar.dma_start(v1all[:], v1d[:])
            u3_t = P.tile([3, LH], F32, name="u3t", tag="u3t")
            nc.sync.dma_start(u3_t[:], u3d[:])
            w3_t = P.tile([3, L2], F32, name="w3t", tag="w3t")
            nc.sync.dma_start(w3_t[:], w3d[:])

            o = 0
            def view(n):
                nonlocal o
                v = big_t[:, o:o + n]
                o += n
                return v
            xa = view(LH + L2)
            cws_t = view(RF)
            ident_t = view(A)
            maskbs_t = [view(L2) for _ in range(2)]
            v2_t = [view(VD) for _ in range(4)]
            v1_t = [v1all[:, t_ * VD:(t_ + 1) * VD] for t_ in range(4)]
            k8s_t = [k8_t[:, (2 * r) * (LH + L2):(2 * r + 1) * (LH + L2)] for r in range(RF)]
            k8c_t = [k8_t[:, (2 * r + 1) * (LH + L2):(2 * r + 2) * (LH + L2)] for r in range(RF)]
            halfpi = P.tile([A, 1], F32, name="halfpi", tag="halfpi")
            nc.vector.memset(halfpi[:], float(np.pi / 2))

            # ---------- PE warm-up: HAM needs ~3.4us of activity to unthrottle;
            # the PE is otherwise idle during the input-DMA ramp ----------
            warm_t = P.tile([A, L2], BF16, name="warm_t", tag="warm_t")
            nc.gpsimd.memset(warm_t[:], 0.0)
            wps = PSB.tile([A, L2], F32, name="wps", tag="tp")
            for wi in range(20):
                nc.tensor.matmul(wps[:], warm_t[:, 0:A], warm_t[:],
                                 start=(wi == 0), stop=(wi == 19))

            # ---------- score psum tiles, accumulated over 2*RF+1 matmuls ----------
            score_ps = [PSA.tile([A, L2], F32, name=f"score_ps{i_}", tag=f"score{i_}")
                        for i_ in range(2)]
            # mask term FIRST so exp only waits on the last feature matmul
            for it in range(2):
                nc.tensor.matmul(score_ps[it][:], u3_t[:, it * A:(it + 1) * A],
                                 w3_t[:], start=True, stop=False)

            for r in range(RF):
                s = float(FREQS[r] / TWO_PI)
                # g = (x*s) - k  in [-0.5, 0.5); one fused DVE pass per trig kind
                g_r = RP.tile([A, LH + L2], F32, tag="gred")
                nc.vector.scalar_tensor_tensor(g_r[:], xa[:], s, k8s_t[r][:],
                                               ALU.mult, ALU.subtract)
                gc_r = RP.tile([A, LH + L2], F32, tag="gcred")
                nc.vector.scalar_tensor_tensor(gc_r[:], xa[:], s, k8c_t[r][:],
                                               ALU.mult, ALU.subtract)
                sin_r = FP.tile([A, LH + L2], BF16, tag="sin")
                nc.scalar.activation(sin_r[:], g_r[:], AF.Sin, scale=TWO_PI)
                # cos(wx) = sin(2pi*(t+0.25-kc)) = sin(2pi*gc + pi/2)
                cos_r = FP.tile([A, LH + L2], BF16, tag="cos")
                nc.scalar.activation(cos_r[:], gc_r[:], AF.Sin, bias=halfpi[:], scale=TWO_PI)
                # scale x-side features by c_r * ws (per-partition vector, fp32 math)
                fs = FP.tile([A, LH], BF16, tag="fs")
                nc.vector.tensor_scalar(fs[:], sin_r[:, 0:LH], cws_t[:, r:r + 1], None, ALU.mult)
                fc = FP.tile([A, LH], BF16, tag="fc")
                nc.vector.tensor_scalar(fc[:], cos_r[:, 0:LH], cws_t[:, r:r + 1], None, ALU.mult)
                for it in range(2):
                    last = (r == RF - 1)
                    nc.tensor.matmul(score_ps[it][:], fs[:, it * A:(it + 1) * A],
                                     cos_r[:, LH:LH + L2], start=False, stop=False)
                    nc.tensor.matmul(score_ps[it][:], fc[:, it * A:(it + 1) * A],
                                     sin_r[:, LH:LH + L2], start=False, stop=last)

            # ---------- exp + row stats (no max subtraction needed) ----------
            E2 = [P.tile([A, L2], F32, name=f"E2_{i_}", tag=f"E2_{i_}")
                  for i_ in range(2)]                               # [i, j] layout
            S2 = P.tile([A, 2], F32, name="S2", tag="S2")
            for it in range(2):
                nc.scalar.activation(E2[it][:], score_ps[it][:], AF.Exp,
                                     accum_out=S2[:, it:it + 1])

            # ---------- pairwise AllGather of bf16 exp matrix, ASAP ----------
            E2b = [SP.tile([A, L2], BF16, name=f"E2b{i_}", tag="E2b") for i_ in range(2)]
            for it in range(2):
                nc.vector.tensor_copy(E2b[it][:], E2[it][:])
            eIn = DR.tile([LH, L2], BF16, name="eIn", tag="eIn")
            for it in range(2):
                nc.sync.dma_start(eIn[it * A:(it + 1) * A, :], E2b[it][:])
            eAll = DR.tile([L1, L2], BF16, name="eAll", tag="eAll")
            nc.gpsimd.collective_compute(
                "AllGather", ALU.bypass, replica_groups=groups,
                ins=[eIn.opt()], outs=[eAll.opt()])
            Eall = []
            for t_ in range(4):
                t = P.tile([A, L2], BF16, name=f"Eall{t_}", tag=f"Eall{t_}")
                eng = nc.scalar if t_ % 2 == 0 else nc.sync
                eng.dma_start(t[:], eAll[t_ * A:(t_ + 1) * A, :])
                Eall.append(t)

            # ---------- local work that overlaps the collective ----------
            for it in range(2):
                so = SP.tile([A, L2], F32, tag="so")
                nc.vector.tensor_tensor(so[:], score_ps[it][:], maskbs_t[it][:], ALU.add)
                nc.sync.dma_start(score_o[it * A:(it + 1) * A, :], so[:])

            r2 = P.tile([A, 2], F32, name="r2", tag="r2")
            nc.vector.reciprocal(r2[:], S2[:])
            for it in range(2):
                w2s = SP.tile([A, L2], F32, tag="w2s")
                nc.vector.tensor_scalar(w2s[:], E2[it][:], r2[:, it:it + 1], None, ALU.mult)
                nc.sync.dma_start(w2_o[it * A:(it + 1) * A, :], w2s[:])

            E1T = [P.tile([A, LH], F32, name=f"E1T_{j_}", tag=f"E1T_{j_}")
                   for j_ in range(4)]
            for jt in range(4):
                for it in range(2):
                    tp = PSB.tile([A, A], F32, tag="tp")
                    nc.tensor.transpose(tp[:], E2[it][:, jt * A:(jt + 1) * A], ident_t[:])
                    if (jt + it) % 2 == 0:
                        nc.vector.tensor_copy(E1T[jt][:, it * A:(it + 1) * A], tp[:])
                    else:
                        nc.scalar.copy(E1T[jt][:, it * A:(it + 1) * A], tp[:])

            for it in range(2):
                o2ps = PSB.tile([A, VD], F32, name="o2ps", tag="o2ps", bufs=1)
                for jt in range(4):
                    nc.tensor.matmul(o2ps[:], E1T[jt][:, it * A:(it + 1) * A],
                                     v2_t[jt][:], start=(jt == 0), stop=(jt == 3))
                o2s = SP.tile([A, VD], F32, tag="o2s")
                nc.vector.tensor_scalar(o2s[:], o2ps[:], r2[:, it:it + 1], None, ALU.mult)
                nc.sync.dma_start(o2_o[it * A:(it + 1) * A, :], o2s[:])

            # ---------- after AllGather: S1 ([j-part] layout), w1, o1 ----------
            ones_t = P.tile([A, 1], BF16, name="ones_t", tag="ones_t")
            nc.vector.memset(ones_t[:], 1.0)
            S1 = P.tile([A, 4], F32, name="S1", tag="S1")
            for jt in range(4):
                s1ps = PSB.tile([A, 1], F32, name="s1ps", tag="s1ps", bufs=1)
                for it in range(4):
                    nc.tensor.matmul(s1ps[:], Eall[it][:, jt * A:(jt + 1) * A],
                                     ones_t[:], start=(it == 0), stop=(it == 3))
                nc.vector.tensor_copy(S1[:, jt:jt + 1], s1ps[:])
            r1 = P.tile([A, 4], F32, name="r1", tag="r1")
            nc.vector.reciprocal(r1[:], S1[:])

            for jt in range(4):
                o1ps = PSB.tile([A, VD], F32, name="o1ps", tag="o1ps", bufs=2)
                for it in range(4):
                    nc.tensor.matmul(o1ps[:], Eall[it][:, jt * A:(jt + 1) * A],
                                     v1_t[it][:], start=(it == 0), stop=(it == 3))
                o1s = SP.tile([A, VD], F32, tag="o1s")
                nc.vector.tensor_scalar(o1s[:], o1ps[:], r1[:, jt:jt + 1], None, ALU.mult)
                nc.sync.dma_start(o1_o[jt * A:(jt + 1) * A, :], o1s[:])
                w1s = SP.tile([A, LH], F32, tag="w1s")
                nc.vector.tensor_scalar(w1s[:], E1T[jt][:], r1[:, jt:jt + 1], None, ALU.mult)
                nc.scalar.dma_start(w1_o[jt * A:(jt + 1) * A, :], w1s[:])

    nc.compile()
    return nc


def _get_program():
    if "nc" not in _PROGRAM_CACHE:
        _PROGRAM_CACHE["nc"] = _build_program()
    return _PROGRAM_CACHE["nc"]


def _prep_inputs(k1, k2, v1, v2, W1, b1, W2, b2, ws, bs, k1_lengths, k2_lengths):
    f32 = np.float32
    cws_np = (np.asarray(ws, f32)[:, None] * np.asarray(COEFS, f32)[None, :]).astype(f32)
    W1f = np.asarray(W1, f32)
    W2f = np.asarray(W2, f32)
    # host copies of the projections; only the integer quotients k are derived
    # from these (an off-by-eps shifts the Sin arg by ~2pi*eps -- harmless)
    p1h = [(np.asarray(k1[b_], f32) @ W1f + np.asarray(b1, f32)).astype(f32)
           for b_ in range(B)]
    p2h = [(np.asarray(k2[b_], f32) @ W2f + np.asarray(b2, f32)).astype(f32)
           for b_ in range(B)]
    scales = (np.asarray(FREQS, f32) / f32(2 * np.pi)).astype(f32)
    in_maps = []
    for c in range(N_CORES):
        b_, h_ = c // 2, c % 2
        sl = slice(h_ * LH, (h_ + 1) * LH)
        xa_h = np.concatenate([p1h[b_][sl].T, p2h[b_].T], axis=1).astype(f32)  # [A, LH+L2]
        k8 = np.empty((A, 2 * RF * (LH + L2)), np.int8)
        W_ = LH + L2
        for r in range(RF):
            t = (xa_h * scales[r]).astype(f32)
            k8[:, (2 * r) * W_:(2 * r + 1) * W_] = np.rint(t).astype(np.int8)
            k8[:, (2 * r + 1) * W_:(2 * r + 2) * W_] = np.rint(t + f32(0.25)).astype(np.int8)
        u = (np.arange(L1)[sl] >= int(k1_lengths[b_])).astype(f32)   # [LH]
        w = (np.arange(L2) >= int(k2_lengths[b_])).astype(f32)       # [L2]
        C = f32(-BIGNEG)  # 1e30
        u3 = np.stack([-C * u, -C * np.ones_like(u), 2 * C * u]).astype(f32)
        w3 = np.stack([np.ones_like(w), w, w]).astype(f32)
        mask = (u[:, None] + w[None, :]) == 1.0
        maskbs_np = np.where(mask, f32(-np.inf), f32(bs)).astype(f32)
        # packed fp32 input [A, C_ALL]: xa|cws|ident|maskbs|v2
        v2f = np.asarray(v2[b_], f32)                # [L2, VD]
        parts = [
            xa_h,
            cws_np, np.eye(A, dtype=f32),
            maskbs_np[0:A, :], maskbs_np[A:2 * A, :],
            v2f[0:A, :], v2f[A:2 * A, :], v2f[2 * A:3 * A, :], v2f[3 * A:4 * A, :],
        ]
        bigin = np.ascontiguousarray(np.concatenate(parts, axis=1).astype(f32))
        v1f = np.asarray(v1[b_], f32)                # [L1, VD] full
        v1b = np.ascontiguousarray(np.concatenate(
            [v1f[t_ * A:(t_ + 1) * A, :] for t_ in range(4)], axis=1).astype(_BF16))
        in_maps.append({
            "bigin": bigin,
            "k8": np.ascontiguousarray(k8),
            "v1b": v1b,
            "u3": np.ascontiguousarray(u3),
            "w3": np.ascontiguousarray(w3),
        })
    return in_maps


def _execute(inputs, trace=False):
    from concourse.bass_utils import run_bass_kernel_spmd
    nc = _get_program()
    in_maps = _prep_inputs(**inputs)
    res = run_bass_kernel_spmd(nc, in_maps, list(range(N_CORES)), trace=trace)
    f32 = np.float32
    o1 = np.empty((B, L2, VD), f32)
    o2 = np.empty((B, L1, VD), f32)
    w1 = np.empty((B, L2, L1), f32)
    w2 = np.empty((B, L1, L2), f32)
    score = np.empty((B, L1, L2), f32)
    for c in range(N_CORES):
        b_, h_ = c // 2, c % 2
        sl = slice(h_ * LH, (h_ + 1) * LH)
        r = res.results[c]
        score[b_, sl] = r["score_h"]
        w2[b_, sl] = r["w2_h"]
        o2[b_, sl] = r["o2_h"]
        w1[b_, :, sl] = r["w1_h"]
        if h_ == 0:
            o1[b_] = r["o1_h"]  # both cores compute full o1; take h=0's
    return (o1, o2, w1, w2, score), res


def kernel(**inputs):
    outs, _ = _execute(inputs, trace=False)
    return outs
